# revision 36
# baseline (speedup 1.0000x reference)
"""Trainium2 Bass kernel for nn_AttentionBlockManual (dense transformer block).

Reference computation (per batch element n):
    temb = relu(t @ W_t.T + b_t)                      # [C]
    xin  = x + temb[:, None, None]                    # [C, H, W]
    tokens: full spatial attention over L = H*W = 1024 tokens, dim C = 256
    q/k/v = proj(xin), scores = q k^T / 16, P = softmax, o = P v
    out  = o @ Wp.T + bp, transposed back, + residual x

Token relabeling note: the reference's transpose(1,3) is a pure permutation of
the 1024 tokens applied consistently to q/k/v and inverted on output; full
softmax attention is permutation-equivariant, so we use the natural memory
order (h-major) token index and skip both transposes.

Sharding: data-parallel over batch N=32 across 8 cores (4 batches per core),
params replicated. No collectives.

fp8 formulation (all big matmuls run e4m3/e5m2 in DoubleRow perf mode, which
contracts 256 per instruction at 2x rate):
  - scores: S = xin^T (Wq^T Wk) xin, so M = Wq^T Wk is precomputed once
    (bf16 matmul of the naturally-laid-out weights -- no transposes needed)
    and a single fused projection g = M^T xin replaces both Q and K.
    S^T tile [j, i] = matmul(lhsT=x8[:, :, jt], rhs=g8[:, :, ih]).
  - exp: ACT, fp32 PSUM in -> e5m2 out (range fits e5m2; no max-subtraction
    needed), unnormalized.
  - V path is computed from x WITHOUT temb ("centered"): v and the attention
    output then have zero batch-constant component, which is what makes
    e4m3 quantization of Wv/Wp/otn accurate (a fixed relative error on the
    large temb-driven constant otherwise dominates).  The dropped constant
    contributes Wp @ (Wv @ temb) to every token, exact in bf16, and is
    folded into the output bias: out = proj(otn) + (bp + pc) + x.
  - rowsums via an all-ones e4m3 [128,2,128] DoubleRow matmul (replicated
    across partitions, so 1/rowsum needs no cross-partition broadcast);
    normalization via reciprocal_approx_fast at the O^T PSUM->SBUF mul.
  Measured end-to-end rel err vs the fp32 reference: ~7e-3.

Engine/queue balance per batch (~10-11us/batch steady state):
  - ACT: 16 exps + one vt8 drain; DVE: g8/vt8 drains, x8 adds, merged otn
    normalize (0-stride broadcast recip AP), reciprocal, output epilogue.
  - PE: 56 DoubleRow matmuls.  gpsimd tensor ops are ~16x slower than DVE
    (measured on HW; the CoreSim model claims parity -- don't trust it), so
    the Pool engine only runs SWDGE: store DMAs, the x8c = e4(x) CASTING
    loads (batches 1+; re-reads x from HBM -- DMA has slack, DVE/ACT don't),
    and the Wq/Wk fp32->bf16 casting loads.
  - x/x8c are prefetched TWO batches ahead so the x8 adds never stall the
    DVE queue at a batch boundary.
  - t/b_t/bp arrive via natural-layout DMA + tiny identity matmuls on the
    PE (an element-strided transpose DMA costs ~10us of descriptor time).
  - startup: W_t + x0's first token-half lead the sync/scalar queues so the
    temb chain, x8(0) and the first scores start ~5us earlier; batch 0's
    ih0 runs un-flushed (pend depth 4) because its V chain lands late.

The score loop is software-pipelined one even/odd j-tile pair deep, carried
across the query-half boundary (the PE queue is in-order).  Tail work
(recip/otn/proj/epilogue/store) and the next batch's g/V projections are
interleaved into the score loop's emission so they execute under the
ACT-paced exp cadence instead of serializing between batches.
"""

from contextlib import ExitStack

import numpy as np

import concourse.bacc as bacc
import concourse.tile as tile
from concourse import mybir
from concourse.bass_types import AP
from concourse.bass_utils import run_bass_kernel_spmd
from concourse.masks import make_identity


def _bcast2(ap, n):
    """Broadcast a [P, F] AP to [P, n, F] via a 0-stride middle dim."""
    return AP(ap.tensor, ap.offset, [list(ap.ap[0]), [0, n], list(ap.ap[1])])

F32 = mybir.dt.float32
BF16 = mybir.dt.bfloat16
E4 = mybir.dt.float8e4
E5 = mybir.dt.float8e5
AF = mybir.ActivationFunctionType
ALU = mybir.AluOpType
DR = mybir.MatmulPerfMode.DoubleRow

N_CORES = 8
B = 4            # batches per core
C = 256          # channels
L = 1024         # tokens (H*W)
D = 256          # qk/v dim
T = 512          # time embedding dim
P = 128          # partitions
CT = C // P      # 2 channel chunks
DT = D // P      # 2 dim chunks
TT = T // P      # 4 time chunks
JT = L // P      # 8 key-token chunks
JP = JT // 2     # 4 key-token chunk PAIRS (DoubleRow granularity)
NH = 512         # moving-dim chunk (one PSUM bank of fp32)
IH = L // NH     # 2 query-token halves
SCALE = 1.0 / np.sqrt(256.0)


def _build_body(tc, x_d, t_d, wt_d, bt_d, wq_d, wk_d, wv_d, wp_d, bp_d, out_d):
    nc = tc.nc

    ctx = ExitStack()
    const = ctx.enter_context(tc.tile_pool(name="const", bufs=1))
    wraw = ctx.enter_context(tc.tile_pool(name="wraw", bufs=2))
    xpool = ctx.enter_context(tc.tile_pool(name="xp", bufs=3))
    x8pool = ctx.enter_context(tc.tile_pool(name="x8p", bufs=2))
    gpool = ctx.enter_context(tc.tile_pool(name="gp", bufs=2))
    vpool = ctx.enter_context(tc.tile_pool(name="vp", bufs=2))
    espool = ctx.enter_context(tc.tile_pool(name="es", bufs=12))
    otnp = ctx.enter_context(tc.tile_pool(name="otn", bufs=2))
    rbp = ctx.enter_context(tc.tile_pool(name="rb", bufs=2))
    ypool = ctx.enter_context(tc.tile_pool(name="yp", bufs=4))
    # PSUM: 5 shared 1-bank slots (scores/g/V/proj -- the deep ring lets the
    # S stream run ~2 pairs ahead of the exps), 1-bank rowsum, 2-bank O^T.
    # rs/ot are single-buffered: the ih1 accumulation group's first write is
    # emitted after ih0's recip/otn reads, so reuse is safe by emission order.
    pss = ctx.enter_context(tc.tile_pool(name="pss", bufs=5, space="PSUM"))
    rsp = ctx.enter_context(tc.tile_pool(name="rsp", bufs=1, space="PSUM"))
    psot = ctx.enter_context(tc.tile_pool(name="psot", bufs=1, space="PSUM"))

    # ---- constants FIRST: the identity masks are built by gpsimd, and
    # emitting them before the gpsimd DMA issues keeps every PE transpose
    # off the DMA critical path.
    ident = const.tile([P, P], F32, tag="ident")
    make_identity(nc, ident)
    id4 = const.tile([B, B], F32, tag="id4")
    make_identity(nc, id4)
    id2 = const.tile([CT, CT], F32, tag="id2")
    make_identity(nc, id2)
    ones8 = const.tile([P, 2, P], E4, tag="ones8")
    nc.vector.memset(ones8, 1.0)

    # ---- input DMAs ----------------------------------------------------
    # ~100 GB/s effective per queue, so order each queue by when the data
    # gates compute: temb chain needs W_t chunk 0 + t + b_t first; x0 and
    # Wq/Wk gate batch 0's x8/g; Wv by the first PV; Wp/bp only by +10us.
    t_nat = wraw.tile([B, T], F32, tag="tnat")
    wt_raw = wraw.tile([P, CT, T], F32, tag="wtr")
    bt_nat = wraw.tile([CT, P], F32, tag="btn")
    bp_nat = wraw.tile([CT, P], F32, tag="bpn")
    # Wq/Wk land as bf16 via gpsimd casting DMAs (only SWDGE can cast):
    # skips the fp32 staging tiles AND the DVE bf16 casts.
    wq_bf = const.tile([P, DT, C], BF16, tag="wqbf")
    wk_bf = const.tile([P, DT, C], BF16, tag="wkbf")
    x0_sb = xpool.tile([P, CT, L], F32, tag="x", bufs=3)

    # Startup queue order is the critical path: W_t gates the temb chain
    # (longest); x0's first token-half gates x8/g8/first scores; wv/wp gate
    # only batch 0's V / proj prep (~6-10us in).  x in token-halves so the
    # first scores can start while the second half is still in flight.
    NHB = L // 2
    xs0 = x_d[0, 0:P, :, :].rearrange("c h w -> c (h w)")
    xs1 = x_d[0, P:2 * P, :, :].rearrange("c h w -> c (h w)")
    # Startup is raw-DMA-bandwidth-bound (~340 GB/s aggregate, shared by
    # all three rings at packet granularity) and every dma_start ISSUES
    # immediately (the issuing engines have nothing else to do), so the
    # critical set (W_t -> temb; x0-h0 -> x8; Wq/Wk -> M) is spread evenly
    # across all three rings, and everything non-critical (wv, wp, x(1),
    # the x8c casting loads) is throttled behind dummy-tile WAR gates so it
    # doesn't steal bandwidth from the critical window.
    # dummy generations for DMA throttling (see _gate below)
    wq_gate = wraw.tile([P, 1], F32, tag="wqn", bufs=1, name="wq_gate")
    wk_gate = wraw.tile([P, 1], F32, tag="wkn", bufs=1, name="wk_gate")
    wv_gate = wraw.tile([P, 1], F32, tag="wvr", bufs=1, name="wv_gate")
    wp_gate = wraw.tile([P, 1], F32, tag="wpr", bufs=1, name="wp_gate")
    x_gate0 = xpool.tile([P, 1], F32, tag="x", bufs=3, name="x_gate0")
    x_gate1 = xpool.tile([P, 1], F32, tag="x", bufs=3, name="x_gate1")
    x_gate2 = xpool.tile([P, 1], F32, tag="x", bufs=3, name="x_gate2")
    x8c_gate = x8pool.tile([P, 1], E4, tag="x8c", bufs=2, name="x8c_gate")
    gate_scratch = const.tile([P, 1], F32, tag="gscratch")
    for g in (wq_gate, wk_gate, wv_gate, wp_gate, x_gate0, x_gate1, x_gate2,
              x8c_gate):
        nc.vector.memset(g, 0.0)

    def _gate(dummy):
        """Read a dummy tile so the NEXT same-tag allocation's DMA carries a
        WAR wait on this read -- a time-throttle for DMA issue."""
        nc.vector.tensor_copy(out=gate_scratch, in_=dummy[:, 0:1])

    # Phase 1 (ungated): only W_t + tiny tensors -- W_t heads the longest
    # dependency chain (wtT transposes -> temb -> x8 -> g8 -> scores), so
    # it gets the full DMA bandwidth and lands ~2us in.
    nc.sync.dma_start(out=t_nat, in_=t_d)
    nc.sync.dma_start(out=wt_raw[:, 0, :], in_=wt_d[0:P, :])
    nc.sync.dma_start(out=wt_raw[:, 1, :], in_=wt_d[P:2 * P, :])
    nc.gpsimd.dma_start(out=bt_nat, in_=bt_d.rearrange("(a p) -> a p", p=P))
    nc.gpsimd.dma_start(out=bp_nat, in_=bp_d.rearrange("(a p) -> a p", p=P))

    # Phase 2 (released when W_t has landed): x0 + Wq/Wk, spread evenly.
    nc.vector.tensor_copy(out=gate_scratch, in_=wt_raw[:, 0, 0:1])
    nc.vector.tensor_copy(out=gate_scratch, in_=wt_raw[:, 1, 0:1])
    _gate(x_gate0)
    _gate(wq_gate)
    _gate(wk_gate)
    wq_nat = wraw.tile([P, DT, C], F32, tag="wqn", bufs=1, name="wq_nat")
    wk_nat = wraw.tile([P, DT, C], F32, tag="wkn", bufs=1, name="wk_nat")

    nc.sync.dma_start(out=x0_sb[:, 0, 0:NHB], in_=xs0[:, 0:NHB])
    nc.sync.dma_start(out=wq_nat[:, 0, :], in_=wq_d[0:P, :])
    nc.sync.dma_start(out=x0_sb[:, 0, NHB:L], in_=xs0[:, NHB:L])

    nc.scalar.dma_start(out=x0_sb[:, 1, 0:NHB], in_=xs1[:, 0:NHB])
    nc.scalar.dma_start(out=wq_nat[:, 1, :], in_=wq_d[P:2 * P, :])
    nc.scalar.dma_start(out=x0_sb[:, 1, NHB:L], in_=xs1[:, NHB:L])

    nc.gpsimd.dma_start(out=wk_nat[:, 0, :], in_=wk_d[0:P, :])
    nc.gpsimd.dma_start(out=wk_nat[:, 1, :], in_=wk_d[P:2 * P, :])

    def load_wv():
        _gate(wv_gate)
        wv = wraw.tile([P, DT, C], F32, tag="wvr", bufs=1, name="wv_real")
        nc.sync.dma_start(out=wv[:, 0, :], in_=wv_d[0:P, :])
        nc.scalar.dma_start(out=wv[:, 1, :], in_=wv_d[P:2 * P, :])
        return wv

    def load_wp():
        _gate(wp_gate)
        wp = wraw.tile([P, CT, D], F32, tag="wpr", bufs=1, name="wp_real")
        nc.sync.dma_start(out=wp[:, 0, :], in_=wp_d[0:P, :])
        nc.scalar.dma_start(out=wp[:, 1, :], in_=wp_d[P:2 * P, :])
        return wp

    # ---- small transposes via identity matmuls -------------------------
    # t [B, T] -> t_all_bf [128, TT, B];  b_t/bp [CT, 128] -> [128, CT]
    t_all_bf = const.tile([P, TT, B], BF16, tag="tallbf")
    for kt in range(TT):
        ps = pss.tile([P, B], F32, tag="ps")
        nc.tensor.matmul(ps, t_nat[:, kt * P:(kt + 1) * P], id4,
                         start=True, stop=True)
        nc.vector.tensor_copy(out=t_all_bf[:, kt, :], in_=ps)
    bt_sb = const.tile([P, CT], F32, tag="bt")
    bp_sb = const.tile([P, CT], F32, tag="bp")
    for src, dst in ((bt_nat, bt_sb), (bp_nat, bp_sb)):
        ps = pss.tile([P, CT], F32, tag="ps")
        nc.tensor.matmul(ps, src, id2, start=True, stop=True)
        nc.vector.tensor_copy(out=dst, in_=ps)

    # ---- temb chain first (it gates batch 0's x8) ----------------------
    # copies split ACT/DVE: measured faster than all-on-ACT (the ACT queue
    # is the longer pole at startup -- it also carries batch 0's x8 casts).
    wtT = const.tile([P, TT, C], BF16, tag="wtT")
    for a in range(CT):
        for b in range(TT):
            ps = pss.tile([P, P], F32, tag="ps")
            nc.tensor.transpose(ps, wt_raw[:, a, b * P:(b + 1) * P], ident)
            if (a * TT + b) % 2 == 0:
                nc.scalar.copy(out=wtT[:, b, a * P:(a + 1) * P], in_=ps)
            else:
                nc.vector.tensor_copy(out=wtT[:, b, a * P:(a + 1) * P], in_=ps)
    # temb for all batches + bf16 copy (for the pc matmuls)
    temb_all = const.tile([P, CT, B], F32, tag="temba")
    temb_bf = const.tile([P, CT, B], BF16, tag="tembbf")
    for ct in range(CT):
        tb_ps = pss.tile([P, B], F32, tag="ps")
        for kt in range(TT):
            nc.tensor.matmul(tb_ps, wtT[:, kt, ct * P:(ct + 1) * P],
                             t_all_bf[:, kt, :], start=(kt == 0), stop=(kt == TT - 1))
        nc.scalar.activation(out=temb_all[:, ct, :], in_=tb_ps, func=AF.Relu,
                             bias=bt_sb[:, ct:ct + 1], scale=1.0)
        nc.vector.tensor_copy(out=temb_bf[:, ct, :], in_=temb_all[:, ct, :])

    # ---- M = Wq^T Wk  (bf16 matmul; M rows on partitions) --------------
    nc.vector.tensor_copy(out=wq_bf, in_=wq_nat)
    nc.vector.tensor_copy(out=wk_bf, in_=wk_nat)
    m8 = const.tile([P, CT, C], E4, tag="m8")
    for cm in range(CT):
        ps = pss.tile([P, C], F32, tag="ps")
        for kd in range(DT):
            nc.tensor.matmul(ps, wq_bf[:, kd, cm * P:(cm + 1) * P],
                             wk_bf[:, kd, :], start=(kd == 0), stop=(kd == DT - 1))
        nc.vector.tensor_copy(out=m8[:, cm, :], in_=ps)

    # ---- deferred weight prep: emitted into batch 0's score loop -------
    wvT_bf = const.tile([P, CT, D], BF16, tag="wvTbf")
    wv8T = const.tile([P, CT, D], E4, tag="wv8T")
    wpT_bf = const.tile([P, DT, C], BF16, tag="wpTbf")
    wp8T = const.tile([P, DT, C], E4, tag="wp8T")
    vtmp_bf = const.tile([P, DT, B], BF16, tag="vtmp")
    pcb_all = const.tile([P, CT, B], F32, tag="pcb")

    def emit_wv_prep(wv_raw):
        for a in range(DT):
            for b in range(CT):
                ps = pss.tile([P, P], F32, tag="ps")
                nc.tensor.transpose(ps, wv_raw[:, a, b * P:(b + 1) * P], ident)
                nc.scalar.copy(out=wvT_bf[:, b, a * P:(a + 1) * P], in_=ps)
                nc.vector.tensor_copy(out=wv8T[:, b, a * P:(a + 1) * P], in_=ps)

    def emit_wp_prep(wp_raw):
        for a in range(CT):
            for b in range(DT):
                ps = pss.tile([P, P], F32, tag="ps")
                nc.tensor.transpose(ps, wp_raw[:, a, b * P:(b + 1) * P], ident)
                nc.scalar.copy(out=wpT_bf[:, b, a * P:(a + 1) * P], in_=ps)
                nc.vector.tensor_copy(out=wp8T[:, b, a * P:(a + 1) * P], in_=ps)

    def emit_vtmp():
        for m in range(DT):
            ps = pss.tile([P, B], F32, tag="ps")
            for kc in range(CT):
                nc.tensor.matmul(ps, wvT_bf[:, kc, m * P:(m + 1) * P],
                                 temb_bf[:, kc, :], start=(kc == 0),
                                 stop=(kc == CT - 1))
            nc.vector.tensor_copy(out=vtmp_bf[:, m, :], in_=ps)

    def emit_pcb():
        """pcb = bp + Wp @ (Wv @ temb): the centered-V add-back bias."""
        for ct in range(CT):
            ps = pss.tile([P, B], F32, tag="ps")
            for kd in range(DT):
                nc.tensor.matmul(ps, wpT_bf[:, kd, ct * P:(ct + 1) * P],
                                 vtmp_bf[:, kd, :], start=(kd == 0),
                                 stop=(kd == DT - 1))
            nc.vector.tensor_scalar_add(pcb_all[:, ct, :], ps, bp_sb[:, ct:ct + 1])

    # ---- per-batch pipeline pieces --------------------------------------
    state = {0: dict(x_sb=x0_sb)}

    def load_x8c(n):
        """x8c = e4(x) straight off the wire: SWDGE casting DMA, issued two
        batches ahead (re-reads x from HBM; DMA has slack, DVE/ACT don't)."""
        if n >= B:
            return
        x8c = x8pool.tile([P, CT, L], E4, tag="x8c", bufs=2)
        for ct in range(CT):
            nc.gpsimd.dma_start(
                out=x8c[:, ct, :],
                in_=x_d[n, ct * P:(ct + 1) * P, :, :].rearrange("c h w -> c (h w)"))
        state[n] = dict(x8c=x8c)

    def load_x(n):
        """fp32 x, for the epilogue residual only -- issued 1.5 batches
        ahead (split across both HWDGE rings)."""
        if n >= B:
            return
        x_sb = xpool.tile([P, CT, L], F32, tag="x", bufs=3)
        nc.sync.dma_start(
            out=x_sb[:, 0, :],
            in_=x_d[n, 0:P, :, :].rearrange("c h w -> c (h w)"))
        nc.scalar.dma_start(
            out=x_sb[:, 1, :],
            in_=x_d[n, P:2 * P, :, :].rearrange("c h w -> c (h w)"))
        state[n]["x_sb"] = x_sb

    def emit_x8c0(half):
        """Batch 0 only: x8c = e4(x) on the engines (the SWDGE cast for
        batch 0 would sit behind the weight loads on the gpsimd queue).
        Emitted in token-halves as x0 lands."""
        s = state[0]
        x_sb = s["x_sb"]
        if half == 0:
            s["x8c"] = x8pool.tile([P, CT, L], E4, tag="x8c", bufs=2, name="x8c0")
        x8c = s["x8c"]
        hsl = slice(half * NHB, (half + 1) * NHB)
        nc.scalar.copy(out=x8c[:, 0, hsl], in_=x_sb[:, 0, hsl])
        nc.vector.tensor_copy(out=x8c[:, 1, hsl], in_=x_sb[:, 1, hsl])

    def emit_x80(half):
        """Batch 0: x8 token-half direct from x (ACT bias-add for ct0, DVE
        for ct1) -- skips the x8c hop on the startup critical path."""
        s = state[0]
        x_sb = s["x_sb"]
        if half == 0:
            s["x8"] = x8pool.tile([P, CT, L], E4, tag="x8", name="x80")
        x8 = s["x8"]
        hsl = slice(half * NHB, (half + 1) * NHB)
        nc.scalar.activation(out=x8[:, 0, hsl], in_=x_sb[:, 0, hsl],
                             func=AF.Identity, bias=temb_all[:, 0, 0:1])
        nc.vector.tensor_scalar_add(x8[:, 1, hsl], x_sb[:, 1, hsl],
                                    temb_all[:, 1, 0:1])

    def emit_x8(n):
        """x8 = x8c + temb_n (all-fp8 DVE ops); x8c arrives via cast-DMA."""
        if n >= B:
            return
        s = state[n]
        x8 = x8pool.tile([P, CT, L], E4, tag="x8")
        x8c = s["x8c"]
        for ct in range(CT):
            nc.vector.tensor_scalar_add(x8[:, ct, :], x8c[:, ct, :],
                                        temb_all[:, ct, n:n + 1])
        s["x8"] = x8

    def emit_g(n, cm, nhs=(0, 1)):
        """Fused q/k projection g = M^T xin (one of CT column chunks)."""
        if n >= B:
            return
        s = state[n]
        if cm == 0 and 0 in nhs:
            g8_new = gpool.tile([P, CT, L], E4, tag="g8")
            s["g8"] = g8_new
        g8 = s["g8"]
        for nh in nhs:
            ps = pss.tile([P, NH], F32, tag="ps")
            nc.tensor.matmul(ps, m8[:, :, cm * P:(cm + 1) * P],
                             s["x8"][:, :, nh * NH:(nh + 1) * NH],
                             start=True, stop=True, perf_mode=DR)
            if cm == 1 and nh == 1 and n > 0:
                nc.scalar.copy(out=g8[:, cm, nh * NH:(nh + 1) * NH], in_=ps)
            else:
                nc.vector.tensor_copy(out=g8[:, cm, nh * NH:(nh + 1) * NH], in_=ps)

    def emit_v(n, jp):
        """Centered V^T, one jt-pair (2 matmuls sharing one PSUM bank).
        Drains alternate DVE/ACT to balance the engines (jp==1 on ACT)."""
        if n >= B:
            return
        s = state[n]
        if jp == 0:
            vt8_new = vpool.tile([P, JT, D], E4, tag="vt8")
            s["vt8"] = vt8_new
        vt8 = s["vt8"]
        ps = pss.tile([P, NH], F32, tag="ps")
        for half in range(2):
            jt = 2 * jp + half
            nc.tensor.matmul(ps[:, half * D:(half + 1) * D],
                             s["x8c"][:, :, jt * P:(jt + 1) * P], wv8T,
                             start=True, stop=True, perf_mode=DR)
        if jp == 1:
            nc.scalar.copy(out=vt8[:, 2 * jp:2 * jp + 2, :], in_=ps)
        else:
            nc.vector.tensor_copy(out=vt8[:, 2 * jp:2 * jp + 2, :], in_=ps)

    def emit_recip_otn(n, ih):
        s = state[n]
        recip_b = rbp.tile([P, NH], F32, tag="recipb")
        nc.vector.reciprocal_approx_fast(out=recip_b, in_=s["rs_list"][ih])
        otn = otnp.tile([P, DT, NH], E4, tag="otn")
        # single DVE op over both dh chunks; recip broadcast via 0-stride AP
        nc.vector.tensor_mul(otn[:, :, :], s["ot_list"][ih][:, :, :],
                             _bcast2(recip_b[:, :], DT))
        s.setdefault("otn", {})[ih] = otn

    def emit_tails_pe(n, ih):
        """Projection + epilogue + store for one query half."""
        s = state[n]
        isl = slice(ih * NH, (ih + 1) * NH)
        otn = s["otn"][ih]
        for ct in range(CT):
            pj_ps = pss.tile([P, NH], F32, tag="ps")
            nc.tensor.matmul(pj_ps, wp8T[:, :, ct * P:(ct + 1) * P], otn,
                             start=True, stop=True, perf_mode=DR)
            y = ypool.tile([P, NH], F32, tag="y")
            nc.vector.scalar_tensor_tensor(
                out=y, in0=pj_ps, scalar=pcb_all[:, ct, n:n + 1],
                in1=s["x_sb"][:, ct, isl], op0=ALU.add, op1=ALU.add,
            )
            # stores go out on the pool SWDGE queue (keeps sync/scalar free
            # for loads/exps) -- except the LAST batch, whose stores would
            # otherwise drain serially after the compute: sync/scalar are
            # idle by then and all exps are already queued ahead.  Those
            # final stores are split into quarter-tiles across both queues
            # so the last transfer is as short as possible.
            dst = (out_d[n, ct * P:(ct + 1) * P, :, :]
                   .rearrange("c h w -> c (h w)")[:, isl])
            if n == B - 1:
                hh = NH // 2
                for q in range(2):
                    (nc.sync if q == 0 else nc.scalar).dma_start(
                        out=dst[:, q * hh:(q + 1) * hh],
                        in_=y[:, q * hh:(q + 1) * hh],
                    )
            else:
                nc.gpsimd.dma_start(out=dst, in_=y)

    def emit_rs_pv(n, ih, jp, es):
        s = state[n]
        vt8 = s["vt8"]
        nc.tensor.matmul(s["rs_list"][ih], ones8, es,
                         start=(jp == 0), stop=(jp == JP - 1), perf_mode=DR)
        for dh in range(DT):
            nc.tensor.matmul(
                s["ot_list"][ih][:, dh, :],
                vt8[:, 2 * jp:2 * jp + 2, dh * P:(dh + 1) * P],
                es, start=(jp == 0), stop=(jp == JP - 1), perf_mode=DR,
            )

    # The pend list of un-flushed (ih, jp, es) pairs is SHARED across
    # batches: steady-state depth is 4 (a full ih of lag), so recip/otn for
    # an ih closes one full ih later, and tails slide one ih further.  The
    # emission-order rules (rs/ot single-buffer WAR; vt8 written before its
    # first flush; recip after an ih's last flush) all hold by the slot map
    # below.  Batch 0 doesn't flush at all (its V chain lands late); the
    # backlog drains at up to 2 pops per slot during batch 1.
    pend = []

    def flush(limit):
        npop = 0
        while pend and len(pend) > limit and npop < 2:
            emit_rs_pv(*pend.pop(0))
            npop += 1

    def emit_scores(n):
        s = state[n]
        x8, g8 = s["x8"], s["g8"]
        rs_list, ot_list = [], []
        for _ih in range(IH):
            rs_ps = rsp.tile([P, NH], F32, tag="rs")
            ot_ps = psot.tile([P, DT, NH], F32, tag="ot")
            rs_list.append(rs_ps)
            ot_list.append(ot_ps)
        s["rs_list"], s["ot_list"] = rs_list, ot_list

        for ih in range(IH):
            isl = slice(ih * NH, (ih + 1) * NH)
            for jp in range(JP):
                es = espool.tile([P, 2, NH], E5, tag="es")
                for half in range(2):
                    jt = 2 * jp + half
                    st_ps = pss.tile([P, NH], F32, tag="ps")
                    nc.tensor.matmul(st_ps, x8[:, :, jt * P:(jt + 1) * P],
                                     g8[:, :, isl], start=True, stop=True,
                                     perf_mode=DR)
                    nc.scalar.activation(out=es[:, half, :], in_=st_ps,
                                         func=AF.Exp, scale=SCALE)
                pend.append((n, ih, jp, es))
                if n == B - 1 and ih == 1:
                    flush(2)
                else:
                    flush(99 if (n == 0 and ih == 0) else 4)
                if n == 0:
                    # cold start: everything beyond the critical set lands
                    # via throttled DMAs; prep work trails the data.
                    if ih == 0 and jp == 0:
                        emit_x8c0(0)
                        wp_sb[0] = load_wp()
                    if ih == 0 and jp == 1:
                        emit_x80(1)
                        _gate(x_gate1)
                    if ih == 0 and jp == 2:
                        emit_x8c0(1)
                        emit_g(0, 0, (1,))
                    if ih == 0 and jp == 3:
                        emit_g(0, 1, (1,))
                        emit_wv_prep(wv_sb[0])
                        emit_v(0, 0)
                        emit_v(0, 1)
                    if ih == 1 and jp == 0:
                        load_x(1)
                        emit_v(0, 2)
                        emit_v(0, 3)
                        # x8c ring bufs=2: this DMA reuses x8c(0)'s buffer,
                        # so it must be emitted after v(0,3), its last reader
                        load_x8c(2)
                    if ih == 1 and jp == 1:
                        emit_x8(1)
                    if ih == 1 and jp == 2:
                        emit_wp_prep(wp_sb[0])
                    if ih == 1 and jp == 3:
                        emit_g(1, 0)
                        emit_g(1, 1)
                        emit_vtmp()
                else:
                    if ih == 0 and jp == 0:
                        if n == 1:
                            emit_pcb()
                            _gate(x_gate2)
                    if ih == 0 and jp == 1:
                        emit_x8(n + 1)
                        emit_tails_pe(n - 1, 0)
                        if n > 1:
                            emit_v(n, 3)
                    if ih == 0 and jp == 2:
                        if n == 1:
                            emit_v(1, 0)
                        else:
                            # after v(n,3), its buffer-partner's last reader
                            load_x8c(n + 2)
                    if ih == 0 and jp == 3:
                        emit_recip_otn(n - 1, 1)
                        if n == 1:
                            emit_v(1, 1)
                    if ih == 1 and jp == 0:
                        load_x(n + 1)
                        emit_tails_pe(n - 1, 1)
                        emit_g(n + 1, 0)
                        if n == 1:
                            emit_v(1, 2)
                    if ih == 1 and jp == 1:
                        emit_v(n + 1, 0)
                        if n == 1:
                            emit_v(1, 3)
                    if ih == 1 and jp == 2:
                        emit_g(n + 1, 1)
                        emit_v(n + 1, 1)
                        if n == 1:
                            load_x8c(3)
                    if ih == 1 and jp == 3:
                        emit_recip_otn(n, 0)
                        emit_v(n + 1, 2)
                        if n == B - 1:
                            emit_tails_pe(n, 0)
        if n == 0:
            # ih0's rowsum closed at the ih1-jp3 pop; recip/otn(0,0) must be
            # emitted before batch 1's first pop overwrites the rs/ot banks.
            emit_recip_otn(0, 0)

    wv_sb = {}
    wp_sb = {}
    emit_x80(0)
    emit_g(0, 0, (0,))
    emit_g(0, 1, (0,))
    wv_sb[0] = load_wv()
    _gate(x8c_gate)
    load_x8c(1)
    for n in range(B):
        emit_scores(n)
    while pend:
        emit_rs_pv(*pend.pop(0))
    emit_recip_otn(B - 1, 1)
    emit_tails_pe(B - 1, 1)

    ctx.close()


_CACHE = {}


def _get_program():
    if "nc" in _CACHE:
        return _CACHE["nc"]
    nc = bacc.Bacc("TRN2", target_bir_lowering=False, debug=False,
                   num_devices=N_CORES)
    x_d = nc.dram_tensor("x", [B, C, 32, 32], F32, kind="ExternalInput").ap()
    t_d = nc.dram_tensor("t", [B, T], F32, kind="ExternalInput").ap()
    wt_d = nc.dram_tensor("W_t", [C, T], F32, kind="ExternalInput").ap()
    bt_d = nc.dram_tensor("b_t", [C], F32, kind="ExternalInput").ap()
    wq_d = nc.dram_tensor("Wq", [D, C], F32, kind="ExternalInput").ap()
    wk_d = nc.dram_tensor("Wk", [D, C], F32, kind="ExternalInput").ap()
    wv_d = nc.dram_tensor("Wv", [D, C], F32, kind="ExternalInput").ap()
    wp_d = nc.dram_tensor("Wp", [C, D], F32, kind="ExternalInput").ap()
    bp_d = nc.dram_tensor("bp", [C], F32, kind="ExternalInput").ap()
    out_d = nc.dram_tensor("out", [B, C, 32, 32], F32, kind="ExternalOutput").ap()

    with tile.TileContext(nc) as tc:
        _build_body(tc, x_d, t_d, wt_d, bt_d, wq_d, wk_d, wv_d, wp_d, bp_d, out_d)
    nc.compile()
    _CACHE["nc"] = nc
    return nc


def _run(inputs, trace=False, tmpdir=None):
    nc = _get_program()
    x = np.ascontiguousarray(np.asarray(inputs["x"], dtype=np.float32))
    t = np.ascontiguousarray(np.asarray(inputs["t"], dtype=np.float32))
    rep = {
        k: np.ascontiguousarray(np.asarray(inputs[k], dtype=np.float32))
        for k in ("W_t", "b_t", "Wq", "Wk", "Wv", "Wp", "bp")
    }
    in_maps = []
    for i in range(N_CORES):
        m = {"x": x[i * B:(i + 1) * B], "t": t[i * B:(i + 1) * B]}
        m.update(rep)
        in_maps.append(m)
    res = run_bass_kernel_spmd(nc, in_maps, list(range(N_CORES)),
                               trace=trace, tmpdir=tmpdir)
    out = np.concatenate([res.results[i]["out"] for i in range(N_CORES)], axis=0)
    return out, res


def kernel(**inputs):
    out, _ = _run(inputs)
    return out



# revision 39
# speedup vs baseline: 1.0390x; 1.0390x over previous
"""Trainium2 Bass kernel for nn_AttentionBlockManual (dense transformer block).

Reference computation (per batch element n):
    temb = relu(t @ W_t.T + b_t)                      # [C]
    xin  = x + temb[:, None, None]                    # [C, H, W]
    tokens: full spatial attention over L = H*W = 1024 tokens, dim C = 256
    q/k/v = proj(xin), scores = q k^T / 16, P = softmax, o = P v
    out  = o @ Wp.T + bp, transposed back, + residual x

Token relabeling note: the reference's transpose(1,3) is a pure permutation of
the 1024 tokens applied consistently to q/k/v and inverted on output; full
softmax attention is permutation-equivariant, so we use the natural memory
order (h-major) token index and skip both transposes.

Sharding: data-parallel over batch N=32 across 8 cores (4 batches per core),
params replicated. No collectives.

fp8 formulation (all big matmuls run e4m3/e5m2 in DoubleRow perf mode, which
contracts 256 per instruction at 2x rate):
  - scores: S = xin^T (Wq^T Wk) xin, so M = Wq^T Wk is precomputed once
    (bf16 matmul of the naturally-laid-out weights -- no transposes needed)
    and a single fused projection g = M^T xin replaces both Q and K.
    S^T tile [j, i] = matmul(lhsT=x8[:, :, jt], rhs=g8[:, :, ih]).
  - exp: ACT, fp32 PSUM in -> e5m2 out (range fits e5m2; no max-subtraction
    needed), unnormalized.
  - V path is computed from x WITHOUT temb ("centered"): v and the attention
    output then have zero batch-constant component, which is what makes
    e4m3 quantization of Wv/Wp/otn accurate (a fixed relative error on the
    large temb-driven constant otherwise dominates).  The dropped constant
    contributes Wp @ (Wv @ temb) to every token, exact in bf16, and is
    folded into the output bias: out = proj(otn) + (bp + pc) + x.
  - rowsums via an all-ones e4m3 [128,2,128] DoubleRow matmul (replicated
    across partitions, so 1/rowsum needs no cross-partition broadcast);
    normalization via reciprocal_approx_fast at the O^T PSUM->SBUF mul.
  Measured end-to-end rel err vs the fp32 reference: ~7e-3.

Engine/queue balance per batch (~10-11us/batch steady state):
  - ACT: 16 exps + one vt8 drain; DVE: g8/vt8 drains, x8 adds, merged otn
    normalize (0-stride broadcast recip AP), reciprocal, output epilogue.
  - PE: 56 DoubleRow matmuls.  gpsimd tensor ops are ~16x slower than DVE
    (measured on HW; the CoreSim model claims parity -- don't trust it), so
    the Pool engine only runs SWDGE: store DMAs, the x8c = e4(x) CASTING
    loads (batches 1+; re-reads x from HBM -- DMA has slack, DVE/ACT don't),
    and the Wq/Wk fp32->bf16 casting loads.
  - x/x8c are prefetched TWO batches ahead so the x8 adds never stall the
    DVE queue at a batch boundary.
  - t/b_t/bp arrive via natural-layout DMA + tiny identity matmuls on the
    PE (an element-strided transpose DMA costs ~10us of descriptor time).
  - startup: W_t + x0's first token-half lead the sync/scalar queues so the
    temb chain, x8(0) and the first scores start ~5us earlier; batch 0's
    ih0 runs un-flushed (pend depth 4) because its V chain lands late.

The score loop is software-pipelined one even/odd j-tile pair deep, carried
across the query-half boundary (the PE queue is in-order).  Tail work
(recip/otn/proj/epilogue/store) and the next batch's g/V projections are
interleaved into the score loop's emission so they execute under the
ACT-paced exp cadence instead of serializing between batches.
"""

from contextlib import ExitStack

import numpy as np

import concourse.bacc as bacc
import concourse.tile as tile
from concourse import mybir
from concourse.bass_types import AP
from concourse.bass_utils import run_bass_kernel_spmd
from concourse.masks import make_identity


def _bcast2(ap, n):
    """Broadcast a [P, F] AP to [P, n, F] via a 0-stride middle dim."""
    return AP(ap.tensor, ap.offset, [list(ap.ap[0]), [0, n], list(ap.ap[1])])

F32 = mybir.dt.float32
BF16 = mybir.dt.bfloat16
E4 = mybir.dt.float8e4
E5 = mybir.dt.float8e5
AF = mybir.ActivationFunctionType
ALU = mybir.AluOpType
DR = mybir.MatmulPerfMode.DoubleRow

N_CORES = 8
B = 4            # batches per core
C = 256          # channels
L = 1024         # tokens (H*W)
D = 256          # qk/v dim
T = 512          # time embedding dim
P = 128          # partitions
CT = C // P      # 2 channel chunks
DT = D // P      # 2 dim chunks
TT = T // P      # 4 time chunks
JT = L // P      # 8 key-token chunks
JP = JT // 2     # 4 key-token chunk PAIRS (DoubleRow granularity)
NH = 512         # moving-dim chunk (one PSUM bank of fp32)
IH = L // NH     # 2 query-token halves
SCALE = 1.0 / np.sqrt(256.0)


def _build_body(tc, x_d, t_d, wt_d, bt_d, wq_d, wk_d, wv_d, wp_d, bp_d, out_d):
    nc = tc.nc

    ctx = ExitStack()
    const = ctx.enter_context(tc.tile_pool(name="const", bufs=1))
    wraw = ctx.enter_context(tc.tile_pool(name="wraw", bufs=2))
    xpool = ctx.enter_context(tc.tile_pool(name="xp", bufs=3))
    x8pool = ctx.enter_context(tc.tile_pool(name="x8p", bufs=2))
    gpool = ctx.enter_context(tc.tile_pool(name="gp", bufs=2))
    vpool = ctx.enter_context(tc.tile_pool(name="vp", bufs=2))
    espool = ctx.enter_context(tc.tile_pool(name="es", bufs=12))
    otnp = ctx.enter_context(tc.tile_pool(name="otn", bufs=2))
    rbp = ctx.enter_context(tc.tile_pool(name="rb", bufs=2))
    ypool = ctx.enter_context(tc.tile_pool(name="yp", bufs=4))
    # PSUM: 5 shared 1-bank slots (scores/g/V/proj -- the deep ring lets the
    # S stream run ~2 pairs ahead of the exps), 1-bank rowsum, 2-bank O^T.
    # rs/ot are single-buffered: the ih1 accumulation group's first write is
    # emitted after ih0's recip/otn reads, so reuse is safe by emission order.
    pss = ctx.enter_context(tc.tile_pool(name="pss", bufs=5, space="PSUM"))
    rsp = ctx.enter_context(tc.tile_pool(name="rsp", bufs=1, space="PSUM"))
    psot = ctx.enter_context(tc.tile_pool(name="psot", bufs=1, space="PSUM"))

    # ---- constants FIRST: the identity masks are built by gpsimd, and
    # emitting them before the gpsimd DMA issues keeps every PE transpose
    # off the DMA critical path.
    ident = const.tile([P, P], F32, tag="ident")
    make_identity(nc, ident)
    id4 = const.tile([B, B], F32, tag="id4")
    make_identity(nc, id4)
    id2 = const.tile([CT, CT], F32, tag="id2")
    make_identity(nc, id2)
    ones8 = const.tile([P, 2, P], E4, tag="ones8")
    nc.vector.memset(ones8, 1.0)

    # ---- input DMAs ----------------------------------------------------
    # ~100 GB/s effective per queue, so order each queue by when the data
    # gates compute: temb chain needs W_t chunk 0 + t + b_t first; x0 and
    # Wq/Wk gate batch 0's x8/g; Wv by the first PV; Wp/bp only by +10us.
    t_nat = wraw.tile([B, T], F32, tag="tnat")
    wt_raw = wraw.tile([P, CT, T], F32, tag="wtr")
    bt_nat = wraw.tile([CT, P], F32, tag="btn")
    bp_nat = wraw.tile([CT, P], F32, tag="bpn")
    # Wq/Wk land as bf16 via gpsimd casting DMAs (only SWDGE can cast):
    # skips the fp32 staging tiles AND the DVE bf16 casts.
    wq_bf = const.tile([P, DT, C], BF16, tag="wqbf")
    wk_bf = const.tile([P, DT, C], BF16, tag="wkbf")
    x0_sb = xpool.tile([P, CT, L], F32, tag="x", bufs=3)

    # Startup queue order is the critical path: W_t gates the temb chain
    # (longest); x0's first token-half gates x8/g8/first scores; wv/wp gate
    # only batch 0's V / proj prep (~6-10us in).  x in token-halves so the
    # first scores can start while the second half is still in flight.
    NHB = L // 2
    xs0 = x_d[0, 0:P, :, :].rearrange("c h w -> c (h w)")
    xs1 = x_d[0, P:2 * P, :, :].rearrange("c h w -> c (h w)")
    # Startup is raw-DMA-bandwidth-bound (~340 GB/s aggregate, shared by
    # all three rings at packet granularity) and every dma_start ISSUES
    # immediately (the issuing engines have nothing else to do), so the
    # critical set (W_t -> temb; x0-h0 -> x8; Wq/Wk -> M) is spread evenly
    # across all three rings, and everything non-critical (wv, wp, x(1),
    # the x8c casting loads) is throttled behind dummy-tile WAR gates so it
    # doesn't steal bandwidth from the critical window.
    # dummy generations for DMA throttling (see _gate below)
    wq_gate = wraw.tile([P, 1], F32, tag="wqn", bufs=1, name="wq_gate")
    wk_gate = wraw.tile([P, 1], F32, tag="wkn", bufs=1, name="wk_gate")
    wv_gate = wraw.tile([P, 1], F32, tag="wvr", bufs=1, name="wv_gate")
    wp_gate = wraw.tile([P, 1], F32, tag="wpr", bufs=1, name="wp_gate")
    x_gate0 = xpool.tile([P, 1], F32, tag="x", bufs=3, name="x_gate0")
    x_gate1 = xpool.tile([P, 1], F32, tag="x", bufs=3, name="x_gate1")
    x_gate2 = xpool.tile([P, 1], F32, tag="x", bufs=3, name="x_gate2")
    x8c_gate = x8pool.tile([P, 1], E4, tag="x8c", bufs=2, name="x8c_gate")
    gate_scratch = const.tile([P, 1], F32, tag="gscratch")
    for g in (wq_gate, wk_gate, wv_gate, wp_gate, x_gate0, x_gate1, x_gate2,
              x8c_gate):
        nc.vector.memset(g, 0.0)

    def _gate(dummy):
        """Read a dummy tile so the NEXT same-tag allocation's DMA carries a
        WAR wait on this read -- a time-throttle for DMA issue."""
        nc.vector.tensor_copy(out=gate_scratch, in_=dummy[:, 0:1])

    # Per-RING bandwidth caps at ~105 GB/s (measured: W_t alone on one ring
    # still takes ~7us), so W_t itself is split across all three rings to
    # land ~4.3us, and the rest of the critical set (x0-h0, Wq/Wk) is
    # packed evenly behind it.  x0-h1 rides the ring tails.
    wq_nat = wraw.tile([P, DT, C], F32, tag="wqn", bufs=1, name="wq_nat")
    wk_nat = wraw.tile([P, DT, C], F32, tag="wkn", bufs=1, name="wk_nat")
    TQ = 384
    nc.sync.dma_start(out=t_nat, in_=t_d)
    nc.sync.dma_start(out=wt_raw[:, 0, 0:TQ], in_=wt_d[0:P, 0:TQ])
    nc.scalar.dma_start(out=wt_raw[:, 0, TQ:T], in_=wt_d[0:P, TQ:T])
    nc.scalar.dma_start(out=wt_raw[:, 1, 0:256], in_=wt_d[P:2 * P, 0:256])
    nc.gpsimd.dma_start(out=bt_nat, in_=bt_d.rearrange("(a p) -> a p", p=P))
    nc.gpsimd.dma_start(out=bp_nat, in_=bp_d.rearrange("(a p) -> a p", p=P))
    nc.gpsimd.dma_start(out=wt_raw[:, 1, 256:T], in_=wt_d[P:2 * P, 256:T])

    nc.sync.dma_start(out=x0_sb[:, 0, 0:NHB], in_=xs0[:, 0:NHB])
    nc.sync.dma_start(out=wq_nat[:, 0, :], in_=wq_d[0:P, :])
    nc.sync.dma_start(out=x0_sb[:, 0, NHB:L], in_=xs0[:, NHB:L])

    nc.scalar.dma_start(out=x0_sb[:, 1, 0:NHB], in_=xs1[:, 0:NHB])
    nc.scalar.dma_start(out=wq_nat[:, 1, :], in_=wq_d[P:2 * P, :])
    nc.scalar.dma_start(out=x0_sb[:, 1, NHB:L], in_=xs1[:, NHB:L])

    nc.gpsimd.dma_start(out=wk_nat[:, 0, :], in_=wk_d[0:P, :])
    nc.gpsimd.dma_start(out=wk_nat[:, 1, :], in_=wk_d[P:2 * P, :])

    def load_wv():
        _gate(wv_gate)
        wv = wraw.tile([P, DT, C], F32, tag="wvr", bufs=1, name="wv_real")
        nc.sync.dma_start(out=wv[:, 0, :], in_=wv_d[0:P, :])
        nc.scalar.dma_start(out=wv[:, 1, :], in_=wv_d[P:2 * P, :])
        return wv

    def load_wp():
        _gate(wp_gate)
        wp = wraw.tile([P, CT, D], F32, tag="wpr", bufs=1, name="wp_real")
        nc.sync.dma_start(out=wp[:, 0, :], in_=wp_d[0:P, :])
        nc.scalar.dma_start(out=wp[:, 1, :], in_=wp_d[P:2 * P, :])
        return wp

    # ---- small transposes via identity matmuls -------------------------
    # t [B, T] -> t_all_bf [128, TT, B];  b_t/bp [CT, 128] -> [128, CT]
    t_all_bf = const.tile([P, TT, B], BF16, tag="tallbf")
    for kt in range(TT):
        ps = pss.tile([P, B], F32, tag="ps")
        nc.tensor.matmul(ps, t_nat[:, kt * P:(kt + 1) * P], id4,
                         start=True, stop=True)
        nc.vector.tensor_copy(out=t_all_bf[:, kt, :], in_=ps)
    bt_sb = const.tile([P, CT], F32, tag="bt")
    bp_sb = const.tile([P, CT], F32, tag="bp")
    for src, dst in ((bt_nat, bt_sb), (bp_nat, bp_sb)):
        ps = pss.tile([P, CT], F32, tag="ps")
        nc.tensor.matmul(ps, src, id2, start=True, stop=True)
        nc.vector.tensor_copy(out=dst, in_=ps)

    # ---- temb chain first (it gates batch 0's x8) ----------------------
    # copies split ACT/DVE: measured faster than all-on-ACT (the ACT queue
    # is the longer pole at startup -- it also carries batch 0's x8 casts).
    wtT = const.tile([P, TT, C], BF16, tag="wtT")
    for a in range(CT):
        for b in range(TT):
            ps = pss.tile([P, P], F32, tag="ps")
            nc.tensor.transpose(ps, wt_raw[:, a, b * P:(b + 1) * P], ident)
            if (a * TT + b) % 2 == 0:
                nc.scalar.copy(out=wtT[:, b, a * P:(a + 1) * P], in_=ps)
            else:
                nc.vector.tensor_copy(out=wtT[:, b, a * P:(a + 1) * P], in_=ps)
    # temb for all batches + bf16 copy (for the pc matmuls)
    temb_all = const.tile([P, CT, B], F32, tag="temba")
    temb_bf = const.tile([P, CT, B], BF16, tag="tembbf")
    for ct in range(CT):
        tb_ps = pss.tile([P, B], F32, tag="ps")
        for kt in range(TT):
            nc.tensor.matmul(tb_ps, wtT[:, kt, ct * P:(ct + 1) * P],
                             t_all_bf[:, kt, :], start=(kt == 0), stop=(kt == TT - 1))
        nc.scalar.activation(out=temb_all[:, ct, :], in_=tb_ps, func=AF.Relu,
                             bias=bt_sb[:, ct:ct + 1], scale=1.0)
        nc.vector.tensor_copy(out=temb_bf[:, ct, :], in_=temb_all[:, ct, :])

    # ---- M = Wq^T Wk  (bf16 matmul; M rows on partitions) --------------
    nc.vector.tensor_copy(out=wq_bf, in_=wq_nat)
    nc.vector.tensor_copy(out=wk_bf, in_=wk_nat)
    m8 = const.tile([P, CT, C], E4, tag="m8")
    for cm in range(CT):
        ps = pss.tile([P, C], F32, tag="ps")
        for kd in range(DT):
            nc.tensor.matmul(ps, wq_bf[:, kd, cm * P:(cm + 1) * P],
                             wk_bf[:, kd, :], start=(kd == 0), stop=(kd == DT - 1))
        nc.vector.tensor_copy(out=m8[:, cm, :], in_=ps)

    # ---- deferred weight prep: emitted into batch 0's score loop -------
    wvT_bf = const.tile([P, CT, D], BF16, tag="wvTbf")
    wv8T = const.tile([P, CT, D], E4, tag="wv8T")
    wpT_bf = const.tile([P, DT, C], BF16, tag="wpTbf")
    wp8T = const.tile([P, DT, C], E4, tag="wp8T")
    vtmp_bf = const.tile([P, DT, B], BF16, tag="vtmp")
    pcb_all = const.tile([P, CT, B], F32, tag="pcb")

    def emit_wv_prep(wv_raw):
        for a in range(DT):
            for b in range(CT):
                ps = pss.tile([P, P], F32, tag="ps")
                nc.tensor.transpose(ps, wv_raw[:, a, b * P:(b + 1) * P], ident)
                nc.scalar.copy(out=wvT_bf[:, b, a * P:(a + 1) * P], in_=ps)
                nc.vector.tensor_copy(out=wv8T[:, b, a * P:(a + 1) * P], in_=ps)

    def emit_wp_prep(wp_raw):
        for a in range(CT):
            for b in range(DT):
                ps = pss.tile([P, P], F32, tag="ps")
                nc.tensor.transpose(ps, wp_raw[:, a, b * P:(b + 1) * P], ident)
                nc.scalar.copy(out=wpT_bf[:, b, a * P:(a + 1) * P], in_=ps)
                nc.vector.tensor_copy(out=wp8T[:, b, a * P:(a + 1) * P], in_=ps)

    def emit_vtmp():
        for m in range(DT):
            ps = pss.tile([P, B], F32, tag="ps")
            for kc in range(CT):
                nc.tensor.matmul(ps, wvT_bf[:, kc, m * P:(m + 1) * P],
                                 temb_bf[:, kc, :], start=(kc == 0),
                                 stop=(kc == CT - 1))
            nc.vector.tensor_copy(out=vtmp_bf[:, m, :], in_=ps)

    def emit_pcb():
        """pcb = bp + Wp @ (Wv @ temb): the centered-V add-back bias."""
        for ct in range(CT):
            ps = pss.tile([P, B], F32, tag="ps")
            for kd in range(DT):
                nc.tensor.matmul(ps, wpT_bf[:, kd, ct * P:(ct + 1) * P],
                                 vtmp_bf[:, kd, :], start=(kd == 0),
                                 stop=(kd == DT - 1))
            nc.vector.tensor_scalar_add(pcb_all[:, ct, :], ps, bp_sb[:, ct:ct + 1])

    # ---- per-batch pipeline pieces --------------------------------------
    state = {0: dict(x_sb=x0_sb)}

    def load_x8c(n):
        """x8c = e4(x) straight off the wire: SWDGE casting DMA, issued two
        batches ahead (re-reads x from HBM; DMA has slack, DVE/ACT don't)."""
        if n >= B:
            return
        x8c = x8pool.tile([P, CT, L], E4, tag="x8c", bufs=2)
        for ct in range(CT):
            nc.gpsimd.dma_start(
                out=x8c[:, ct, :],
                in_=x_d[n, ct * P:(ct + 1) * P, :, :].rearrange("c h w -> c (h w)"))
        state[n] = dict(x8c=x8c)

    def load_x(n):
        """fp32 x, for the epilogue residual only -- issued 1.5 batches
        ahead (split across both HWDGE rings)."""
        if n >= B:
            return
        x_sb = xpool.tile([P, CT, L], F32, tag="x", bufs=3)
        nc.sync.dma_start(
            out=x_sb[:, 0, :],
            in_=x_d[n, 0:P, :, :].rearrange("c h w -> c (h w)"))
        nc.scalar.dma_start(
            out=x_sb[:, 1, :],
            in_=x_d[n, P:2 * P, :, :].rearrange("c h w -> c (h w)"))
        state[n]["x_sb"] = x_sb

    def emit_x8c0(half):
        """Batch 0 only: x8c = e4(x) on the engines (the SWDGE cast for
        batch 0 would sit behind the weight loads on the gpsimd queue).
        Emitted in token-halves as x0 lands."""
        s = state[0]
        x_sb = s["x_sb"]
        if half == 0:
            s["x8c"] = x8pool.tile([P, CT, L], E4, tag="x8c", bufs=2, name="x8c0")
        x8c = s["x8c"]
        hsl = slice(half * NHB, (half + 1) * NHB)
        nc.scalar.copy(out=x8c[:, 0, hsl], in_=x_sb[:, 0, hsl])
        nc.vector.tensor_copy(out=x8c[:, 1, hsl], in_=x_sb[:, 1, hsl])

    def emit_x80(half):
        """Batch 0: x8 token-half direct from x (ACT bias-add for ct0, DVE
        for ct1) -- skips the x8c hop on the startup critical path."""
        s = state[0]
        x_sb = s["x_sb"]
        if half == 0:
            s["x8"] = x8pool.tile([P, CT, L], E4, tag="x8", name="x80")
        x8 = s["x8"]
        hsl = slice(half * NHB, (half + 1) * NHB)
        nc.scalar.activation(out=x8[:, 0, hsl], in_=x_sb[:, 0, hsl],
                             func=AF.Identity, bias=temb_all[:, 0, 0:1])
        nc.vector.tensor_scalar_add(x8[:, 1, hsl], x_sb[:, 1, hsl],
                                    temb_all[:, 1, 0:1])

    def emit_x8(n):
        """x8 = x8c + temb_n (all-fp8 DVE ops); x8c arrives via cast-DMA."""
        if n >= B:
            return
        s = state[n]
        x8 = x8pool.tile([P, CT, L], E4, tag="x8")
        x8c = s["x8c"]
        for ct in range(CT):
            nc.vector.tensor_scalar_add(x8[:, ct, :], x8c[:, ct, :],
                                        temb_all[:, ct, n:n + 1])
        s["x8"] = x8

    def emit_g(n, cm, nhs=(0, 1)):
        """Fused q/k projection g = M^T xin (one of CT column chunks)."""
        if n >= B:
            return
        s = state[n]
        if cm == 0 and 0 in nhs:
            g8_new = gpool.tile([P, CT, L], E4, tag="g8")
            s["g8"] = g8_new
        g8 = s["g8"]
        for nh in nhs:
            ps = pss.tile([P, NH], F32, tag="ps")
            nc.tensor.matmul(ps, m8[:, :, cm * P:(cm + 1) * P],
                             s["x8"][:, :, nh * NH:(nh + 1) * NH],
                             start=True, stop=True, perf_mode=DR)
            if cm == 1 and nh == 1 and n > 0:
                nc.scalar.copy(out=g8[:, cm, nh * NH:(nh + 1) * NH], in_=ps)
            else:
                nc.vector.tensor_copy(out=g8[:, cm, nh * NH:(nh + 1) * NH], in_=ps)

    def emit_v(n, jp):
        """Centered V^T, one jt-pair (2 matmuls sharing one PSUM bank).
        Drains alternate DVE/ACT to balance the engines (jp==1 on ACT)."""
        if n >= B:
            return
        s = state[n]
        if jp == 0:
            vt8_new = vpool.tile([P, JT, D], E4, tag="vt8")
            s["vt8"] = vt8_new
        vt8 = s["vt8"]
        ps = pss.tile([P, NH], F32, tag="ps")
        for half in range(2):
            jt = 2 * jp + half
            nc.tensor.matmul(ps[:, half * D:(half + 1) * D],
                             s["x8c"][:, :, jt * P:(jt + 1) * P], wv8T,
                             start=True, stop=True, perf_mode=DR)
        if jp == 1:
            nc.scalar.copy(out=vt8[:, 2 * jp:2 * jp + 2, :], in_=ps)
        else:
            nc.vector.tensor_copy(out=vt8[:, 2 * jp:2 * jp + 2, :], in_=ps)

    def emit_recip_otn(n, ih):
        s = state[n]
        recip_b = rbp.tile([P, NH], F32, tag="recipb")
        nc.vector.reciprocal_approx_fast(out=recip_b, in_=s["rs_list"][ih])
        otn = otnp.tile([P, DT, NH], E4, tag="otn")
        # single DVE op over both dh chunks; recip broadcast via 0-stride AP
        nc.vector.tensor_mul(otn[:, :, :], s["ot_list"][ih][:, :, :],
                             _bcast2(recip_b[:, :], DT))
        s.setdefault("otn", {})[ih] = otn

    def emit_tails_pe(n, ih):
        """Projection + epilogue for one query half (stores deferred: the
        SWDGE ring is capped ~105 GB/s and must carry the x8c casting
        loads, so stores ride sync/scalar -- but a store's y-readiness wait
        would stall the issuing engine's queue, so emit_stores runs a slot
        later when y is long done)."""
        s = state[n]
        isl = slice(ih * NH, (ih + 1) * NH)
        otn = s["otn"][ih]
        ys = []
        for ct in range(CT):
            pj_ps = pss.tile([P, NH], F32, tag="ps")
            nc.tensor.matmul(pj_ps, wp8T[:, :, ct * P:(ct + 1) * P], otn,
                             start=True, stop=True, perf_mode=DR)
            y = ypool.tile([P, NH], F32, tag="y")
            nc.vector.scalar_tensor_tensor(
                out=y, in0=pj_ps, scalar=pcb_all[:, ct, n:n + 1],
                in1=s["x_sb"][:, ct, isl], op0=ALU.add, op1=ALU.add,
            )
            ys.append(y)
        s.setdefault("ys", {})[ih] = ys

    def emit_stores(n, ih, last=False):
        s = state[n]
        isl = slice(ih * NH, (ih + 1) * NH)
        for ct in range(CT):
            y = s["ys"][ih][ct]
            dst = (out_d[n, ct * P:(ct + 1) * P, :, :]
                   .rearrange("c h w -> c (h w)")[:, isl])
            if last:
                # final stores: quarter-tiles across both HWDGE rings so the
                # last transfer is as short as possible
                hh = NH // 2
                for q in range(2):
                    (nc.sync if q == 0 else nc.scalar).dma_start(
                        out=dst[:, q * hh:(q + 1) * hh],
                        in_=y[:, q * hh:(q + 1) * hh],
                    )
            else:
                (nc.sync if ct == 0 else nc.scalar).dma_start(out=dst, in_=y)

    def emit_rs_pv(n, ih, jp, es):
        s = state[n]
        vt8 = s["vt8"]
        nc.tensor.matmul(s["rs_list"][ih], ones8, es,
                         start=(jp == 0), stop=(jp == JP - 1), perf_mode=DR)
        for dh in range(DT):
            nc.tensor.matmul(
                s["ot_list"][ih][:, dh, :],
                vt8[:, 2 * jp:2 * jp + 2, dh * P:(dh + 1) * P],
                es, start=(jp == 0), stop=(jp == JP - 1), perf_mode=DR,
            )

    # The pend list of un-flushed (ih, jp, es) pairs is SHARED across
    # batches: steady-state depth is 4 (a full ih of lag), so recip/otn for
    # an ih closes one full ih later, and tails slide one ih further.  The
    # emission-order rules (rs/ot single-buffer WAR; vt8 written before its
    # first flush; recip after an ih's last flush) all hold by the slot map
    # below.  Batch 0 doesn't flush at all (its V chain lands late); the
    # backlog drains at up to 2 pops per slot during batch 1.
    pend = []

    def flush(limit):
        npop = 0
        while pend and len(pend) > limit and npop < 2:
            emit_rs_pv(*pend.pop(0))
            npop += 1

    def emit_scores(n):
        s = state[n]
        x8, g8 = s["x8"], s["g8"]
        rs_list, ot_list = [], []
        for _ih in range(IH):
            rs_ps = rsp.tile([P, NH], F32, tag="rs")
            ot_ps = psot.tile([P, DT, NH], F32, tag="ot")
            rs_list.append(rs_ps)
            ot_list.append(ot_ps)
        s["rs_list"], s["ot_list"] = rs_list, ot_list

        for ih in range(IH):
            isl = slice(ih * NH, (ih + 1) * NH)
            for jp in range(JP):
                es = espool.tile([P, 2, NH], E5, tag="es")
                for half in range(2):
                    jt = 2 * jp + half
                    st_ps = pss.tile([P, NH], F32, tag="ps")
                    nc.tensor.matmul(st_ps, x8[:, :, jt * P:(jt + 1) * P],
                                     g8[:, :, isl], start=True, stop=True,
                                     perf_mode=DR)
                    nc.scalar.activation(out=es[:, half, :], in_=st_ps,
                                         func=AF.Exp, scale=SCALE)
                pend.append((n, ih, jp, es))
                if n == B - 1 and ih == 1:
                    flush(2)
                else:
                    flush(99 if (n == 0 and ih == 0) else 4)
                if n == 0:
                    # cold start: everything beyond the critical set lands
                    # via throttled DMAs; prep work trails the data.
                    if ih == 0 and jp == 0:
                        emit_x8c0(0)
                        wp_sb[0] = load_wp()
                    if ih == 0 and jp == 1:
                        emit_x80(1)
                        _gate(x_gate1)
                    if ih == 0 and jp == 2:
                        emit_x8c0(1)
                        emit_g(0, 0, (1,))
                    if ih == 0 and jp == 3:
                        emit_g(0, 1, (1,))
                        emit_wv_prep(wv_sb[0])
                        emit_v(0, 0)
                        emit_v(0, 1)
                    if ih == 1 and jp == 0:
                        load_x(1)
                        emit_v(0, 2)
                        emit_v(0, 3)
                        # x8c ring bufs=2: this DMA reuses x8c(0)'s buffer,
                        # so it must be emitted after v(0,3), its last reader
                        load_x8c(2)
                    if ih == 1 and jp == 1:
                        emit_x8(1)
                    if ih == 1 and jp == 2:
                        emit_wp_prep(wp_sb[0])
                    if ih == 1 and jp == 3:
                        emit_g(1, 0)
                        emit_g(1, 1)
                        emit_vtmp()
                else:
                    if ih == 0 and jp == 0:
                        if n == 1:
                            emit_pcb()
                            _gate(x_gate2)
                    if ih == 0 and jp == 1:
                        emit_x8(n + 1)
                        emit_tails_pe(n - 1, 0)
                        if n > 1:
                            emit_v(n, 3)
                    if ih == 0 and jp == 2:
                        if n == 1:
                            emit_v(1, 0)
                        else:
                            # after v(n,3), its buffer-partner's last reader
                            load_x8c(n + 2)
                    if ih == 0 and jp == 3:
                        emit_recip_otn(n - 1, 1)
                        if n == 1:
                            emit_v(1, 1)
                    if ih == 1 and jp == 0:
                        load_x(n + 1)
                        emit_tails_pe(n - 1, 1)
                        emit_g(n + 1, 0)
                        if n == 1:
                            emit_v(1, 2)
                    if ih == 1 and jp == 1:
                        emit_v(n + 1, 0)
                        if n == 1:
                            emit_v(1, 3)
                        if n == B - 1:
                            # ih0's pops finished at this slot's top (the
                            # last batch drains at 2-lag) -- close it early
                            emit_recip_otn(n, 0)
                            emit_tails_pe(n, 0)
                    if ih == 1 and jp == 2:
                        emit_g(n + 1, 1)
                        emit_v(n + 1, 1)
                        if n == 1:
                            load_x8c(3)
                    if ih == 1 and jp == 3:
                        if n < B - 1:
                            emit_recip_otn(n, 0)
                        emit_v(n + 1, 2)
        if n == 0:
            # ih0's rowsum closed at the ih1-jp3 pop; recip/otn(0,0) must be
            # emitted before batch 1's first pop overwrites the rs/ot banks.
            emit_recip_otn(0, 0)

    wv_sb = {}
    wp_sb = {}
    emit_x80(0)
    emit_g(0, 0, (0,))
    emit_g(0, 1, (0,))
    wv_sb[0] = load_wv()
    _gate(x8c_gate)
    load_x8c(1)
    for n in range(B):
        emit_scores(n)
    while pend:
        emit_rs_pv(*pend.pop(0))
    emit_recip_otn(B - 1, 1)
    emit_tails_pe(B - 1, 1)

    ctx.close()


_CACHE = {}


def _get_program():
    if "nc" in _CACHE:
        return _CACHE["nc"]
    nc = bacc.Bacc("TRN2", target_bir_lowering=False, debug=False,
                   num_devices=N_CORES)
    x_d = nc.dram_tensor("x", [B, C, 32, 32], F32, kind="ExternalInput").ap()
    t_d = nc.dram_tensor("t", [B, T], F32, kind="ExternalInput").ap()
    wt_d = nc.dram_tensor("W_t", [C, T], F32, kind="ExternalInput").ap()
    bt_d = nc.dram_tensor("b_t", [C], F32, kind="ExternalInput").ap()
    wq_d = nc.dram_tensor("Wq", [D, C], F32, kind="ExternalInput").ap()
    wk_d = nc.dram_tensor("Wk", [D, C], F32, kind="ExternalInput").ap()
    wv_d = nc.dram_tensor("Wv", [D, C], F32, kind="ExternalInput").ap()
    wp_d = nc.dram_tensor("Wp", [C, D], F32, kind="ExternalInput").ap()
    bp_d = nc.dram_tensor("bp", [C], F32, kind="ExternalInput").ap()
    out_d = nc.dram_tensor("out", [B, C, 32, 32], F32, kind="ExternalOutput").ap()

    with tile.TileContext(nc) as tc:
        _build_body(tc, x_d, t_d, wt_d, bt_d, wq_d, wk_d, wv_d, wp_d, bp_d, out_d)
    nc.compile()
    _CACHE["nc"] = nc
    return nc


def _run(inputs, trace=False, tmpdir=None):
    nc = _get_program()
    x = np.ascontiguousarray(np.asarray(inputs["x"], dtype=np.float32))
    t = np.ascontiguousarray(np.asarray(inputs["t"], dtype=np.float32))
    rep = {
        k: np.ascontiguousarray(np.asarray(inputs[k], dtype=np.float32))
        for k in ("W_t", "b_t", "Wq", "Wk", "Wv", "Wp", "bp")
    }
    in_maps = []
    for i in range(N_CORES):
        m = {"x": x[i * B:(i + 1) * B], "t": t[i * B:(i + 1) * B]}
        m.update(rep)
        in_maps.append(m)
    res = run_bass_kernel_spmd(nc, in_maps, list(range(N_CORES)),
                               trace=trace, tmpdir=tmpdir)
    out = np.concatenate([res.results[i]["out"] for i in range(N_CORES)], axis=0)
    return out, res


def kernel(**inputs):
    out, _ = _run(inputs)
    return out

